# revision 56
# baseline (speedup 1.0000x reference)
"""Trainium2 Bass kernel for nn_Entropy (histogram_binning): per-pixel Shannon
entropy of a 5x5-window KDE histogram over 256 intensity bins.

Math: k(x,b) = sigmoid'(10(x-b)) = s(1-s), s = sigmoid(10(x-b)).
  q[h,w,b] = 5x5 zero-padded window sum of k;  S = sum_b q (analytic taps);
  E = (S*ln(S+eps) - sum_b q*ln(q+eps)) / (S+eps).

Engine split per (img, bin-half) stripe, layout [96h, (w,b)] b-inner (bf16):
  - z = 10x-10b on TensorE: one K=18 matmul per 8-w chunk; stationary =
    [x_hi^T; x_lo^T; ones; ones] (split-bf16 exact), moving = tiny const
    selector carrying 10*onehot and the -10b bias split (16m + r, both
    bf16-exact).
  - s = Sigmoid (ScalarE, PSUM->SBUF), u = Square(s - 0.5) (ScalarE)
    -> written into a w-padded k-stripe; k = 0.25 - u via one in-place
    4x-mode tensor_scalar (VectorE).
  - BOTH window dims on TensorE: 5 shifted accumulating band matmuls
    (band[h',h] = |h-h'|<=2, bf16) over the padded k-stripe -> q in PSUM.
  - backend: L = Ln(q + eps) (ScalarE), e = q*L (VectorE TT), per-w bin
    reduce (VectorE) -> T; finalize E from T and the analytic S path.
Stripes are software-pipelined (front of stripe i+1 overlaps backend of i).
Sharding: B*C = 24 images, 3 per core across 8 cores; no collectives.
"""

import sys

sys.path.insert(0, "/opt/trn_rl_repo")

import numpy as np

H = 96
W = 96
NB = 256
NBH = 128         # bins per stripe (half)
NIMG = 3
NCORES = 8
EPS = 1e-10
CHW = 8           # w's per chunk
NCH = W // CHW    # 12 chunks per stripe
KBLK = W + 4      # w-blocks in padded k stripe (2 pads each side)
KCOLS = KBLK * NBH
DCOLS = W * NBH   # 12288 data cols per stripe
NW = NIMG * W
STT_STRIPES = frozenset((0, 2, 4))  # stripes whose k is built on DVE (else ScalarE)

_CACHE = {}


def _bf16(a):
    import ml_dtypes

    return np.asarray(a, dtype=ml_dtypes.bfloat16)


def _build_consts():
    import ml_dtypes

    # selector moving constants per bin-half: [26, CHW*NBH]
    movs = []
    for half in range(2):
        c = np.zeros((26, CHW * NBH), dtype=np.float64)
        for j in range(CHW):
            c[j, j * NBH:(j + 1) * NBH] = 10.0
            c[j + 8, j * NBH:(j + 1) * NBH] = 10.0
            c[j + 16, j * NBH:(j + 1) * NBH] = 10.0
        b = np.arange(NBH, dtype=np.float64) + half * NBH
        tb = 10.0 * b
        A = 16.0 * np.floor(tb / 16.0)
        Br = tb - A
        c[24, :] = np.tile(-A, CHW)
        c[25, :] = np.tile(-Br, CHW)
        movs.append(_bf16(c))
    hh = np.arange(H)
    band = (np.abs(hh[:, None] - hh[None, :]) <= 2).astype(np.float32)
    # main-path band is NEGATED: k-tile holds -k = (s-1)*s from the gpsimd
    # scalar_tensor_tensor, so q = sum((-1) * (-k)) = sum(k)
    return movs[0], movs[1], _bf16(-band), band


def _emit_spath(nc, tc, sm, xall, bandf_sb, bias_ap):
    """Analytic per-pixel KDE mass S summed over the 5x5 window.
    Returns (swin [H, NIMG, W] f32 window-summed S, rtile = 1/(S+EPS))."""
    from concourse import mybir

    f32 = mybir.dt.float32
    i32 = mybir.dt.int32
    AF = mybir.ActivationFunctionType
    OP = mybir.AluOpType

    ni = sm.tile([H, NW], i32)
    nc.vector.tensor_copy(ni[:], xall[:])
    nf = sm.tile([H, NW], f32)
    nc.vector.tensor_copy(nf[:], ni[:])
    u = sm.tile([H, NW], f32)
    nc.vector.tensor_tensor(u[:], xall[:], nf[:], op=OP.subtract)
    taps = (-2, -1, 0, 1, 2)
    sq = {}
    for o in taps:
        v = sm.tile([H, NW], f32, tag=f"v{o}")
        nc.scalar.activation(v[:], u[:], AF.Tanh, bias=bias_ap(-5.0 * o), scale=5.0)
        s2 = sm.tile([H, NW], f32, tag=f"sq{o}")
        nc.scalar.activation(s2[:], v[:], AF.Square)
        sq[o] = s2
    masks = {}
    for o in taps:
        if o == 0:
            continue
        m = sm.tile([H, NW], f32, tag=f"m{o}")
        if o < 0:
            nc.vector.tensor_scalar(m[:], nf[:], float(-o), None, op0=OP.is_ge)
        else:
            nc.vector.tensor_scalar(m[:], nf[:], float(255 - o), None, op0=OP.is_le)
        masks[o] = m
    cnt = sm.tile([H, NW], f32)
    nc.vector.tensor_tensor(cnt[:], masks[-2][:], masks[-1][:], op=OP.add)
    nc.vector.tensor_tensor(cnt[:], cnt[:], masks[1][:], op=OP.add)
    nc.vector.tensor_tensor(cnt[:], cnt[:], masks[2][:], op=OP.add)
    nc.vector.tensor_scalar(cnt[:], cnt[:], 1.0, None, op0=OP.add)
    ssum = sm.tile([H, NW], f32)
    nc.vector.tensor_copy(ssum[:], sq[0][:])
    for o in (-2, -1, 1, 2):
        t_m = sm.tile([H, NW], f32, tag=f"tm{o}")
        nc.vector.tensor_tensor(t_m[:], masks[o][:], sq[o][:], op=OP.mult)
        nc.vector.tensor_tensor(ssum[:], ssum[:], t_m[:], op=OP.add)
    spix = sm.tile([H, NW], f32)
    nc.vector.tensor_tensor(spix[:], cnt[:], ssum[:], op=OP.subtract)
    nc.vector.tensor_scalar(spix[:], spix[:], 0.25, None, op0=OP.mult)
    return spix


def _emit_kernel(nc, tc, ctx, ins, outs):
    from concourse import mybir

    f32 = mybir.dt.float32
    bf16 = mybir.dt.bfloat16
    AF = mybir.ActivationFunctionType
    OP = mybir.AluOpType

    x_d, xg_d, mov0_d, mov1_d, band_d, bandf_d = ins
    (ent_d,) = outs

    consts = ctx.enter_context(tc.tile_pool(name="consts", bufs=1))
    sm = ctx.enter_context(tc.tile_pool(name="sm", bufs=1))
    kpool = ctx.enter_context(tc.tile_pool(name="kpool", bufs=2))
    p2pool = ctx.enter_context(tc.tile_pool(name="p2pool", bufs=2))
    spool = ctx.enter_context(tc.tile_pool(name="spool", bufs=3))
    upool = ctx.enter_context(tc.tile_pool(name="upool", bufs=3))
    lpool = ctx.enter_context(tc.tile_pool(name="lpool", bufs=3))
    epool = ctx.enter_context(tc.tile_pool(name="epool", bufs=3))
    zpsum = ctx.enter_context(tc.tile_pool(name="zpsum", bufs=2, space="PSUM"))
    qpsum = ctx.enter_context(tc.tile_pool(name="qpsum", bufs=2, space="PSUM"))

    # ---- constants / inputs ----
    # selector replicated at partition bases 0/32/64 to match stationary base
    mov_sb = []
    for half, md in ((0, mov0_d), (1, mov1_d)):
        t = consts.tile([90, CHW * NBH], bf16, tag=f"mov{half}")
        for k3 in range(3):
            nc.sync.dma_start(t[32 * k3:32 * k3 + 26, :], md[:])
        mov_sb.append(t)
    band_sb = consts.tile([H, H], bf16)
    nc.sync.dma_start(band_sb[:], band_d[:])
    bandf_sb = consts.tile([H, H], f32)
    nc.sync.dma_start(bandf_sb[:], bandf_d[:])

    xall = consts.tile([H, NW], f32)
    for i in range(NIMG):
        nc.sync.dma_start(xall[:, i * W:(i + 1) * W], x_d[i])

    # stationary groups: [26 rows: x^T 3-way bf16 split (8 w each); ones x2]
    # packed 3 groups per tile at partition offsets 0/32/64 (LDW constraint),
    # host-assembled and shipped whole
    xg_ap = []  # per group g: (tile, row base)
    for tg in range(4):
        gt = consts.tile([90, NIMG * H], bf16, tag=f"xg{tg}")
        nc.sync.dma_start(gt[:], xg_d[tg])
        for k3 in range(3):
            xg_ap.append((gt, 32 * k3))

    bias_tiles = {}

    def bias_ap(val):
        if val not in bias_tiles:
            t = consts.tile([H, 1], f32, tag=f"bias{val}")
            nc.vector.memset(t[:], val)
            bias_tiles[val] = t
        return bias_tiles[val][:]

    # =====================  analytic S path  =====================
    spix = _emit_spath(nc, tc, sm, xall, bandf_sb, bias_ap)
    ps_s = zpsum.tile([H, 1024], f32, tag="z")
    nc.tensor.matmul(ps_s[:, 0:NW], bandf_sb[:], spix[:], start=True, stop=True)
    sh = sm.tile([H, NW], f32)
    nc.scalar.copy(sh[:], ps_s[:, 0:NW])
    shp = sm.tile([H, NIMG, W + 4], f32)
    nc.vector.memset(shp[:], 0.0)
    for i in range(NIMG):
        nc.vector.tensor_copy(shp[:, i, 2:2 + W], sh[:, i * W:(i + 1) * W])
    swin = sm.tile([H, NIMG, W], f32)
    nc.vector.tensor_tensor(swin[:], shp[:, :, 0:W], shp[:, :, 1:1 + W], op=OP.add)
    for j in (2, 3, 4):
        nc.vector.tensor_tensor(swin[:], swin[:], shp[:, :, j:j + W], op=OP.add)
    sw_flat = swin[:].rearrange("p a b -> p (a b)")
    rtile = sm.tile([H, NW], f32)
    nc.vector.tensor_scalar(rtile[:], sw_flat, EPS, None, op0=OP.add)
    nc.vector.reciprocal(rtile[:], rtile[:])
    lnS = sm.tile([H, NW], f32)
    nc.scalar.activation(lnS[:], sw_flat, AF.Ln, bias=bias_ap(EPS))
    slns = sm.tile([H, NW], f32)
    nc.vector.tensor_tensor(slns[:], sw_flat, lnS[:], op=OP.mult)

    # =====================  main pipeline  =====================
    # T accumulator per half
    QL0 = sm.tile([H, NW], f32, tag="QL0")
    QL1 = sm.tile([H, NW], f32, tag="QL1")
    QL = [QL0, QL1]
    stripe_store = {}
    # tiny ScalarE barrier tiles forcing [sigmoid-batch][ln-batch] alternation
    # so the act-table isn't thrashed (Copy/MemsetZero live in every set)
    eps_bar = {}   # end of front(s): [H,1] tile holding EPS
    zero_bar = {}  # end of backend(s): [H,1] tile holding 0.0

    def front(s):
        i, half = s // 2, s % 2
        kt = kpool.tile([H, KCOLS], bf16, tag="kt")
        nc.vector.memset(kt[:, 0:2 * NBH], 0.0)
        nc.vector.memset(kt[:, KCOLS - 2 * NBH:], 0.0)
        for c in range(NCH):
            gt, base = xg_ap[c]
            zp = zpsum.tile([H, 1024], f32, tag="z")
            for p in range(2):
                nc.tensor.matmul(
                    zp[:, 512 * p:512 * (p + 1)],
                    gt[base:base + 26, i * H:(i + 1) * H],
                    mov_sb[half][base:base + 26, 512 * p:512 * (p + 1)],
                    start=True, stop=True,
                )
            sc = spool.tile([H, 1024], f32, tag="s")
            sig_bias = zero_bar[s - 2][:] if s - 2 in zero_bar else 0.0
            nc.scalar.activation(sc[:], zp[:], AF.Sigmoid, bias=sig_bias)
            kdst = kt[:, (2 + c * CHW) * NBH:(2 + (c + 1) * CHW) * NBH]
            if s in STT_STRIPES:
                # -k = (s - 1) * s in one DVE pass (f32 in, bf16 out)
                nc.vector.scalar_tensor_tensor(
                    kdst, sc[:], 1.0, sc[:], op0=OP.subtract, op1=OP.mult,
                )
            else:
                # u = (s - 0.5)^2 on ScalarE, then -k = u - 0.25 on DVE (2x)
                uc = upool.tile([H, 1024], f32, tag="u")
                nc.scalar.activation(uc[:], sc[:], AF.Square, bias=bias_ap(-0.5))
                nc.vector.tensor_scalar(
                    kdst, uc[:], 1.0, -0.25, op0=OP.mult, op1=OP.add,
                )
        # P2[wb] = k[wb] + k[wb+1], chunk-granular 2x bf16 DVE passes
        p2 = p2pool.tile([H, (KBLK - 2) * NBH], bf16, tag="p2")
        for c in range(NCH):
            lo = c * CHW * NBH
            nc.vector.tensor_tensor(
                p2[:, lo:lo + CHW * NBH],
                kt[:, lo:lo + CHW * NBH],
                kt[:, lo + NBH:lo + (CHW + 1) * NBH],
                op=OP.add,
            )
        lo = W * NBH
        nc.vector.tensor_tensor(
            p2[:, lo:lo + 2 * NBH],
            kt[:, lo:lo + 2 * NBH],
            kt[:, lo + NBH:lo + 3 * NBH],
            op=OP.add,
        )
        eb = sm.tile([H, 1], f32, tag=f"epsb{s}")
        nc.scalar.activation(eb[:], kt[:, KCOLS - 2 * NBH - 1:KCOLS - 2 * NBH],
                             AF.Copy, scale=0.0, bias=EPS)
        eps_bar[s] = eb
        stripe_store[s] = (kt, p2)

    def backend(s):
        i, half = s // 2, s % 2
        kt, p2 = stripe_store.pop(s)
        ln_bias = eps_bar[s + 1][:] if s + 1 in eps_bar else bias_ap(EPS)
        lc = None
        for c in range(NCH):
            qp = qpsum.tile([H, 1024], f32, tag="q")
            for p in range(2):
                # q[w] = P2[w-2] + P2[w] + k[w+2]  (+2 pad offset in blocks)
                for s0, src in ((0, p2), (2, p2), (4, kt)):
                    base_col = (c * CHW + s0) * NBH + 512 * p
                    nc.tensor.matmul(
                        qp[:, 512 * p:512 * (p + 1)],
                        band_sb[:],
                        src[:, base_col:base_col + 512],
                        start=(s0 == 0), stop=(s0 == 4),
                    )
            lc = lpool.tile([H, 1024], f32, tag="L")
            nc.scalar.activation(lc[:], qp[:], AF.Ln, bias=ln_bias)
            ec = epool.tile([H, 1024], f32, tag="e")
            nc.vector.tensor_tensor(ec[:], qp[:], lc[:], op=OP.mult)
            nc.vector.tensor_reduce(
                QL[half][:, i * W + c * CHW:i * W + (c + 1) * CHW],
                ec[:].rearrange("p (a b) -> p a b", b=NBH),
                axis=mybir.AxisListType.X,
                op=OP.add,
            )
        zb = sm.tile([H, 1], f32, tag=f"zb{s}")
        nc.scalar.activation(zb[:], lc[:, 0:1], AF.Copy, scale=0.0)
        zero_bar[s] = zb

    front(0)
    front(1)
    for s in range(2, 6):
        backend(s - 2)
        front(s)
    backend(4)
    backend(5)

    # E = rtile * (S*ln(S+eps) - T0 - T1)
    ent = sm.tile([H, NW], f32)
    nc.vector.tensor_tensor(ent[:], slns[:], QL[0][:], op=OP.subtract)
    nc.vector.tensor_tensor(ent[:], ent[:], QL[1][:], op=OP.subtract)
    nc.vector.tensor_tensor(ent[:], ent[:], rtile[:], op=OP.mult)
    for i in range(NIMG):
        nc.sync.dma_start(ent_d[i], ent[:, i * W:(i + 1) * W])


def _get_compiled():
    if "nc" in _CACHE:
        return _CACHE["nc"]
    from contextlib import ExitStack

    import concourse.tile as tile
    from concourse import bacc, mybir

    f32 = mybir.dt.float32
    bf16 = mybir.dt.bfloat16
    nc = bacc.Bacc("TRN2", target_bir_lowering=False, debug=False)
    x_d = nc.dram_tensor("x_sh", [NIMG, H, W], f32, kind="ExternalInput").ap()
    xg_d = nc.dram_tensor("xg", [4, 90, NIMG * H], bf16, kind="ExternalInput").ap()
    mov0_d = nc.dram_tensor("mov0", [26, CHW * NBH], bf16, kind="ExternalInput").ap()
    mov1_d = nc.dram_tensor("mov1", [26, CHW * NBH], bf16, kind="ExternalInput").ap()
    band_d = nc.dram_tensor("bandb", [H, H], bf16, kind="ExternalInput").ap()
    bandf_d = nc.dram_tensor("bandf", [H, H], f32, kind="ExternalInput").ap()
    ent_d = nc.dram_tensor("ent", [NIMG, H, W], f32, kind="ExternalOutput").ap()

    with tile.TileContext(nc) as tc:
        with ExitStack() as ctx:
            _emit_kernel(
                nc, tc, ctx,
                (x_d, xg_d, mov0_d, mov1_d, band_d, bandf_d),
                (ent_d,),
            )
    nc.compile()
    _CACHE["nc"] = nc
    return nc


def make_in_maps(x):
    """x: full [8, 3, 96, 96] -> list of 8 per-core input dicts."""
    import ml_dtypes

    x = np.ascontiguousarray(np.asarray(x, dtype=np.float32))
    imgs = x.reshape(NCORES * NIMG, H, W)
    mov0, mov1, bandb, bandf = _build_consts()
    in_maps = []
    for cidx in range(NCORES):
        sh = np.ascontiguousarray(imgs[cidx * NIMG:(cidx + 1) * NIMG])
        xt = sh.transpose(0, 2, 1).transpose(1, 0, 2).reshape(W, NIMG * H)
        # xt[w, i*H+h] = x[i, h, w]
        xt_hi = np.asarray(_bf16(xt), dtype=np.float32)
        xt_mid = np.asarray(_bf16(xt - xt_hi), dtype=np.float32)
        xt_lo = xt - xt_hi - xt_mid
        xg = np.zeros((4, 90, NIMG * H), dtype=np.float32)
        for g in range(12):
            tg, base = g // 3, 32 * (g % 3)
            xg[tg, base:base + 8] = xt_hi[8 * g:8 * g + 8]
            xg[tg, base + 8:base + 16] = xt_mid[8 * g:8 * g + 8]
            xg[tg, base + 16:base + 24] = xt_lo[8 * g:8 * g + 8]
            xg[tg, base + 24:base + 26] = 1.0
        in_maps.append(
            {
                "x_sh": sh,
                "xg": _bf16(xg),
                "mov0": mov0,
                "mov1": mov1,
                "bandb": bandb,
                "bandf": bandf,
            }
        )
    return in_maps


def kernel(x):
    """Full inputs in, full outputs out. x: [8, 3, 96, 96] f32."""
    from concourse.bass_utils import run_bass_kernel_spmd

    nc = _get_compiled()
    in_maps = make_in_maps(x)
    res = run_bass_kernel_spmd(nc, in_maps, list(range(NCORES)))
    out = np.stack([res.results[c]["ent"] for c in range(NCORES)])
    return out.reshape(8, 3, H, W).astype(np.float32)


# revision 58
# speedup vs baseline: 1.1475x; 1.1475x over previous
"""Trainium2 Bass kernel for nn_Entropy (histogram_binning): per-pixel Shannon
entropy of a 5x5-window KDE histogram over 256 intensity bins.

Math: k(x,b) = sigmoid'(10(x-b)) = s(1-s), s = sigmoid(10(x-b)).
  q[h,w,b] = 5x5 zero-padded window sum of k;  S = sum_b q (analytic taps);
  E = (S*ln(S+eps) - sum_b q*ln(q+eps)) / (S+eps).

Engine split per (img, bin-half) stripe, layout [96h, (w,b)] b-inner (bf16):
  - z = 10x-10b on TensorE: one K=18 matmul per 8-w chunk; stationary =
    [x_hi^T; x_lo^T; ones; ones] (split-bf16 exact), moving = tiny const
    selector carrying 10*onehot and the -10b bias split (16m + r, both
    bf16-exact).
  - s = Sigmoid (ScalarE, PSUM->SBUF), u = Square(s - 0.5) (ScalarE)
    -> written into a w-padded k-stripe; k = 0.25 - u via one in-place
    4x-mode tensor_scalar (VectorE).
  - BOTH window dims on TensorE: 5 shifted accumulating band matmuls
    (band[h',h] = |h-h'|<=2, bf16) over the padded k-stripe -> q in PSUM.
  - backend: L = Ln(q + eps) (ScalarE), e = q*L (VectorE TT), per-w bin
    reduce (VectorE) -> T; finalize E from T and the analytic S path.
Stripes are software-pipelined (front of stripe i+1 overlaps backend of i).
Sharding: B*C = 24 images, 3 per core across 8 cores; no collectives.
"""

import sys

sys.path.insert(0, "/opt/trn_rl_repo")

import numpy as np

H = 96
W = 96
NB = 256
NBH = 128         # bins per stripe (half)
NIMG = 3
NCORES = 8
EPS = 1e-10
CHW = 8           # w's per chunk
NCH = W // CHW    # 12 chunks per stripe
KBLK = W + 4      # w-blocks in padded k stripe (2 pads each side)
KCOLS = KBLK * NBH
DCOLS = W * NBH   # 12288 data cols per stripe
NW = NIMG * W
STT_STRIPES = frozenset(range(6))  # stripes whose k is built on DVE (else ScalarE)

_CACHE = {}


def _bf16(a):
    import ml_dtypes

    return np.asarray(a, dtype=ml_dtypes.bfloat16)


def _build_consts():
    import ml_dtypes

    # selector moving constants per bin-half: [26, CHW*NBH]
    movs = []
    for half in range(2):
        c = np.zeros((26, CHW * NBH), dtype=np.float64)
        for j in range(CHW):
            c[j, j * NBH:(j + 1) * NBH] = 10.0
            c[j + 8, j * NBH:(j + 1) * NBH] = 10.0
            c[j + 16, j * NBH:(j + 1) * NBH] = 10.0
        b = np.arange(NBH, dtype=np.float64) + half * NBH
        tb = 10.0 * b
        A = 16.0 * np.floor(tb / 16.0)
        Br = tb - A
        c[24, :] = np.tile(-A, CHW)
        c[25, :] = np.tile(-Br, CHW)
        movs.append(_bf16(c))
    hh = np.arange(H)
    band = (np.abs(hh[:, None] - hh[None, :]) <= 2).astype(np.float32)
    # main-path band is NEGATED: k-tile holds -k = (s-1)*s from the gpsimd
    # scalar_tensor_tensor, so q = sum((-1) * (-k)) = sum(k)
    return movs[0], movs[1], _bf16(-band), band


def _emit_spath(nc, tc, sm, xall, bandf_sb, bias_ap):
    """Analytic per-pixel KDE mass S summed over the 5x5 window.
    Returns (swin [H, NIMG, W] f32 window-summed S, rtile = 1/(S+EPS))."""
    from concourse import mybir

    f32 = mybir.dt.float32
    i32 = mybir.dt.int32
    AF = mybir.ActivationFunctionType
    OP = mybir.AluOpType

    ni = sm.tile([H, NW], i32)
    nc.vector.tensor_copy(ni[:], xall[:])
    nf = sm.tile([H, NW], f32)
    nc.vector.tensor_copy(nf[:], ni[:])
    u = sm.tile([H, NW], f32)
    nc.vector.tensor_tensor(u[:], xall[:], nf[:], op=OP.subtract)
    taps = (-2, -1, 0, 1, 2)
    sq = {}
    for o in taps:
        v = sm.tile([H, NW], f32, tag=f"v{o}")
        nc.scalar.activation(v[:], u[:], AF.Tanh, bias=bias_ap(-5.0 * o), scale=5.0)
        s2 = sm.tile([H, NW], f32, tag=f"sq{o}")
        nc.scalar.activation(s2[:], v[:], AF.Square)
        sq[o] = s2
    masks = {}
    for o in taps:
        if o == 0:
            continue
        m = sm.tile([H, NW], f32, tag=f"m{o}")
        if o < 0:
            nc.vector.tensor_scalar(m[:], nf[:], float(-o), None, op0=OP.is_ge)
        else:
            nc.vector.tensor_scalar(m[:], nf[:], float(255 - o), None, op0=OP.is_le)
        masks[o] = m
    cnt = sm.tile([H, NW], f32)
    nc.vector.tensor_tensor(cnt[:], masks[-2][:], masks[-1][:], op=OP.add)
    nc.vector.tensor_tensor(cnt[:], cnt[:], masks[1][:], op=OP.add)
    nc.vector.tensor_tensor(cnt[:], cnt[:], masks[2][:], op=OP.add)
    nc.vector.tensor_scalar(cnt[:], cnt[:], 1.0, None, op0=OP.add)
    ssum = sm.tile([H, NW], f32)
    nc.vector.tensor_copy(ssum[:], sq[0][:])
    for o in (-2, -1, 1, 2):
        t_m = sm.tile([H, NW], f32, tag=f"tm{o}")
        nc.vector.tensor_tensor(t_m[:], masks[o][:], sq[o][:], op=OP.mult)
        nc.vector.tensor_tensor(ssum[:], ssum[:], t_m[:], op=OP.add)
    spix = sm.tile([H, NW], f32)
    nc.vector.tensor_tensor(spix[:], cnt[:], ssum[:], op=OP.subtract)
    nc.vector.tensor_scalar(spix[:], spix[:], 0.25, None, op0=OP.mult)
    return spix


def _emit_kernel(nc, tc, ctx, ins, outs):
    from concourse import mybir

    f32 = mybir.dt.float32
    bf16 = mybir.dt.bfloat16
    AF = mybir.ActivationFunctionType
    OP = mybir.AluOpType

    x_d, xg_d, mov0_d, mov1_d, band_d, bandf_d = ins
    (ent_d,) = outs

    consts = ctx.enter_context(tc.tile_pool(name="consts", bufs=1))
    sm = ctx.enter_context(tc.tile_pool(name="sm", bufs=1))
    kpool = ctx.enter_context(tc.tile_pool(name="kpool", bufs=2))
    spool = ctx.enter_context(tc.tile_pool(name="spool", bufs=3))
    upool = ctx.enter_context(tc.tile_pool(name="upool", bufs=3))
    lpool = ctx.enter_context(tc.tile_pool(name="lpool", bufs=3))
    epool = ctx.enter_context(tc.tile_pool(name="epool", bufs=3))
    zpsum = ctx.enter_context(tc.tile_pool(name="zpsum", bufs=2, space="PSUM"))
    qpsum = ctx.enter_context(tc.tile_pool(name="qpsum", bufs=2, space="PSUM"))

    # ---- constants / inputs ----
    # selector replicated at partition bases 0/32/64 to match stationary base
    mov_sb = []
    for half, md in ((0, mov0_d), (1, mov1_d)):
        t = consts.tile([90, CHW * NBH], bf16, tag=f"mov{half}")
        for k3 in range(3):
            nc.sync.dma_start(t[32 * k3:32 * k3 + 26, :], md[:])
        mov_sb.append(t)
    band_sb = consts.tile([H, H], bf16)
    nc.sync.dma_start(band_sb[:], band_d[:])
    bandf_sb = consts.tile([H, H], f32)
    nc.sync.dma_start(bandf_sb[:], bandf_d[:])

    xall = consts.tile([H, NW], f32)
    for i in range(NIMG):
        nc.sync.dma_start(xall[:, i * W:(i + 1) * W], x_d[i])

    # stationary groups: [26 rows: x^T 3-way bf16 split (8 w each); ones x2]
    # packed 3 groups per tile at partition offsets 0/32/64 (LDW constraint),
    # host-assembled and shipped whole
    xg_ap = []  # per group g: (tile, row base)
    for tg in range(4):
        gt = consts.tile([90, NIMG * H], bf16, tag=f"xg{tg}")
        nc.sync.dma_start(gt[:], xg_d[tg])
        for k3 in range(3):
            xg_ap.append((gt, 32 * k3))

    bias_tiles = {}

    def bias_ap(val):
        if val not in bias_tiles:
            t = consts.tile([H, 1], f32, tag=f"bias{val}")
            nc.vector.memset(t[:], val)
            bias_tiles[val] = t
        return bias_tiles[val][:]

    # =====================  analytic S path  =====================
    spix = _emit_spath(nc, tc, sm, xall, bandf_sb, bias_ap)
    ps_s = zpsum.tile([H, 1024], f32, tag="z")
    nc.tensor.matmul(ps_s[:, 0:NW], bandf_sb[:], spix[:], start=True, stop=True)
    sh = sm.tile([H, NW], f32)
    nc.scalar.copy(sh[:], ps_s[:, 0:NW])
    shp = sm.tile([H, NIMG, W + 4], f32)
    nc.vector.memset(shp[:], 0.0)
    for i in range(NIMG):
        nc.vector.tensor_copy(shp[:, i, 2:2 + W], sh[:, i * W:(i + 1) * W])
    swin = sm.tile([H, NIMG, W], f32)
    nc.vector.tensor_tensor(swin[:], shp[:, :, 0:W], shp[:, :, 1:1 + W], op=OP.add)
    for j in (2, 3, 4):
        nc.vector.tensor_tensor(swin[:], swin[:], shp[:, :, j:j + W], op=OP.add)
    sw_flat = swin[:].rearrange("p a b -> p (a b)")
    rtile = sm.tile([H, NW], f32)
    nc.vector.tensor_scalar(rtile[:], sw_flat, EPS, None, op0=OP.add)
    nc.vector.reciprocal(rtile[:], rtile[:])
    lnS = sm.tile([H, NW], f32)
    nc.scalar.activation(lnS[:], sw_flat, AF.Ln, bias=bias_ap(EPS))
    slns = sm.tile([H, NW], f32)
    nc.vector.tensor_tensor(slns[:], sw_flat, lnS[:], op=OP.mult)

    # =====================  main pipeline  =====================
    # T accumulator per half
    QL0 = sm.tile([H, NW], f32, tag="QL0")
    QL1 = sm.tile([H, NW], f32, tag="QL1")
    QL = [QL0, QL1]
    stripe_store = {}
    # tiny ScalarE barrier tiles forcing [sigmoid-batch][ln-batch] alternation
    # so the act-table isn't thrashed (Copy/MemsetZero live in every set)
    eps_bar = {}   # end of front(s): [H,1] tile holding EPS
    zero_bar = {}  # end of backend(s): [H,1] tile holding 0.0

    def front(s):
        i, half = s // 2, s % 2
        kt = kpool.tile([H, KCOLS], bf16, tag="kt")
        nc.vector.memset(kt[:, 0:2 * NBH], 0.0)
        nc.vector.memset(kt[:, KCOLS - 2 * NBH:], 0.0)
        for c in range(NCH):
            gt, base = xg_ap[c]
            zp = zpsum.tile([H, 1024], f32, tag="z")
            for p in range(2):
                nc.tensor.matmul(
                    zp[:, 512 * p:512 * (p + 1)],
                    gt[base:base + 26, i * H:(i + 1) * H],
                    mov_sb[half][base:base + 26, 512 * p:512 * (p + 1)],
                    start=True, stop=True,
                )
            sc = spool.tile([H, 1024], f32, tag="s")
            sig_bias = zero_bar[s - 2][:] if s - 2 in zero_bar else 0.0
            nc.scalar.activation(sc[:], zp[:], AF.Sigmoid, bias=sig_bias)
            kdst = kt[:, (2 + c * CHW) * NBH:(2 + (c + 1) * CHW) * NBH]
            if s in STT_STRIPES:
                # -k = (s - 1) * s in one DVE pass (f32 in, bf16 out)
                nc.vector.scalar_tensor_tensor(
                    kdst, sc[:], 1.0, sc[:], op0=OP.subtract, op1=OP.mult,
                )
            else:
                # u = (s - 0.5)^2 on ScalarE, then -k = u - 0.25 on DVE (2x)
                uc = upool.tile([H, 1024], f32, tag="u")
                nc.scalar.activation(uc[:], sc[:], AF.Square, bias=bias_ap(-0.5))
                nc.vector.tensor_scalar(
                    kdst, uc[:], 1.0, -0.25, op0=OP.mult, op1=OP.add,
                )
        eb = sm.tile([H, 1], f32, tag=f"epsb{s}")
        nc.scalar.activation(eb[:], kt[:, KCOLS - 2 * NBH - 1:KCOLS - 2 * NBH],
                             AF.Copy, scale=0.0, bias=EPS)
        eps_bar[s] = eb
        stripe_store[s] = kt

    def backend(s):
        i, half = s // 2, s % 2
        kt = stripe_store.pop(s)
        ln_bias = eps_bar[s + 1][:] if s + 1 in eps_bar else bias_ap(EPS)
        lc = None
        for c in range(NCH):
            qp = qpsum.tile([H, 1024], f32, tag="q")
            for p in range(2):
                for s0 in range(5):
                    base_col = (c * CHW + s0) * NBH + 512 * p
                    nc.tensor.matmul(
                        qp[:, 512 * p:512 * (p + 1)],
                        band_sb[:],
                        kt[:, base_col:base_col + 512],
                        start=(s0 == 0), stop=(s0 == 4),
                    )
            lc = lpool.tile([H, 1024], f32, tag="L")
            nc.scalar.activation(lc[:], qp[:], AF.Ln, bias=ln_bias)
            ec = epool.tile([H, 1024], f32, tag="e")
            nc.vector.tensor_tensor(ec[:], qp[:], lc[:], op=OP.mult)
            nc.vector.tensor_reduce(
                QL[half][:, i * W + c * CHW:i * W + (c + 1) * CHW],
                ec[:].rearrange("p (a b) -> p a b", b=NBH),
                axis=mybir.AxisListType.X,
                op=OP.add,
            )
        zb = sm.tile([H, 1], f32, tag=f"zb{s}")
        nc.scalar.activation(zb[:], lc[:, 0:1], AF.Copy, scale=0.0)
        zero_bar[s] = zb

    front(0)
    front(1)
    for s in range(2, 6):
        backend(s - 2)
        front(s)
    backend(4)
    backend(5)

    # E = rtile * (S*ln(S+eps) - T0 - T1)
    ent = sm.tile([H, NW], f32)
    nc.vector.tensor_tensor(ent[:], slns[:], QL[0][:], op=OP.subtract)
    nc.vector.tensor_tensor(ent[:], ent[:], QL[1][:], op=OP.subtract)
    nc.vector.tensor_tensor(ent[:], ent[:], rtile[:], op=OP.mult)
    for i in range(NIMG):
        nc.sync.dma_start(ent_d[i], ent[:, i * W:(i + 1) * W])


def _get_compiled():
    if "nc" in _CACHE:
        return _CACHE["nc"]
    from contextlib import ExitStack

    import concourse.tile as tile
    from concourse import bacc, mybir

    f32 = mybir.dt.float32
    bf16 = mybir.dt.bfloat16
    nc = bacc.Bacc("TRN2", target_bir_lowering=False, debug=False)
    x_d = nc.dram_tensor("x_sh", [NIMG, H, W], f32, kind="ExternalInput").ap()
    xg_d = nc.dram_tensor("xg", [4, 90, NIMG * H], bf16, kind="ExternalInput").ap()
    mov0_d = nc.dram_tensor("mov0", [26, CHW * NBH], bf16, kind="ExternalInput").ap()
    mov1_d = nc.dram_tensor("mov1", [26, CHW * NBH], bf16, kind="ExternalInput").ap()
    band_d = nc.dram_tensor("bandb", [H, H], bf16, kind="ExternalInput").ap()
    bandf_d = nc.dram_tensor("bandf", [H, H], f32, kind="ExternalInput").ap()
    ent_d = nc.dram_tensor("ent", [NIMG, H, W], f32, kind="ExternalOutput").ap()

    with tile.TileContext(nc) as tc:
        with ExitStack() as ctx:
            _emit_kernel(
                nc, tc, ctx,
                (x_d, xg_d, mov0_d, mov1_d, band_d, bandf_d),
                (ent_d,),
            )
    nc.compile()
    _CACHE["nc"] = nc
    return nc


def make_in_maps(x):
    """x: full [8, 3, 96, 96] -> list of 8 per-core input dicts."""
    import ml_dtypes

    x = np.ascontiguousarray(np.asarray(x, dtype=np.float32))
    imgs = x.reshape(NCORES * NIMG, H, W)
    mov0, mov1, bandb, bandf = _build_consts()
    in_maps = []
    for cidx in range(NCORES):
        sh = np.ascontiguousarray(imgs[cidx * NIMG:(cidx + 1) * NIMG])
        xt = sh.transpose(0, 2, 1).transpose(1, 0, 2).reshape(W, NIMG * H)
        # xt[w, i*H+h] = x[i, h, w]
        xt_hi = np.asarray(_bf16(xt), dtype=np.float32)
        xt_mid = np.asarray(_bf16(xt - xt_hi), dtype=np.float32)
        xt_lo = xt - xt_hi - xt_mid
        xg = np.zeros((4, 90, NIMG * H), dtype=np.float32)
        for g in range(12):
            tg, base = g // 3, 32 * (g % 3)
            xg[tg, base:base + 8] = xt_hi[8 * g:8 * g + 8]
            xg[tg, base + 8:base + 16] = xt_mid[8 * g:8 * g + 8]
            xg[tg, base + 16:base + 24] = xt_lo[8 * g:8 * g + 8]
            xg[tg, base + 24:base + 26] = 1.0
        in_maps.append(
            {
                "x_sh": sh,
                "xg": _bf16(xg),
                "mov0": mov0,
                "mov1": mov1,
                "bandb": bandb,
                "bandf": bandf,
            }
        )
    return in_maps


def kernel(x):
    """Full inputs in, full outputs out. x: [8, 3, 96, 96] f32."""
    from concourse.bass_utils import run_bass_kernel_spmd

    nc = _get_compiled()
    in_maps = make_in_maps(x)
    res = run_bass_kernel_spmd(nc, in_maps, list(range(NCORES)))
    out = np.stack([res.results[c]["ent"] for c in range(NCORES)])
    return out.reshape(8, 3, H, W).astype(np.float32)
